# revision 15
# baseline (speedup 1.0000x reference)
"""AttFusion (attentive CAV fusion) Trainium2 kernel.

Reference semantics per ragged group of n rows (n = record_len[g]):
  tokens q[s, m, :] = x[row m, :, s]  (s over W*H spatial positions)
  score[s, m] = <q[s, 0, :], q[s, m, :]> / sqrt(C)
  w = softmax over m; out[g, :, s] = sum_m w[s, m] * q[s, m, :]

Strategy: data-parallel over spatial positions across 8 NeuronCores
(every core processes all groups over S/8 positions — SPMD-uniform and
load-balanced). Host side transposes each group's rows to
position-major [S_c, n, C] so the device kernel sees SBUF tiles with
partition = position, free = (member, channel). Per 128-position tile:
  - ego score ||e||^2/sqrt(C): ScalarE Square(scale=1/4) with accum_out
  - member scores: one fused DVE scalar_tensor_tensor per member
    (product + free-dim reduce + 1/sqrt(C) scale in one instruction)
  - softmax: ScalarE Exp with accum_out, DVE reciprocal; no
    max-subtraction (scores bounded ~ sqrt(C)+few sigma, far from
    fp32 overflow); weights w = u/Z normalized up front (bf16-safe:
    the dominant ego weight rounds to exactly 1.0)
  - the weighted sum rides the otherwise-idle TensorEngine *flipped*:
    lhsT = X c-half (fp32 stationary), rhs = diag(w_m) (bf16 moving,
    single-pass N=128 matmuls) accumulated over members in PSUM.
    All n diagonals are built in ONE broadcast-AP tensor_tensor
    (identity-replicated x weight-replicated). PSUM holds ctx in
    [c, s] layout = the output layout, so the host-side gather needs
    no transpose. PSUM->SBUF copies are split ScalarE/VectorE.
"""

import math
import os

import numpy as np

_jp = os.environ.get("JAX_PLATFORMS")
if _jp is not None and "axon" not in _jp:
    # The device run goes through the axon PJRT backend; a cpu-only pin
    # (used for reference-side jax) would hide the NeuronCores.
    os.environ.pop("JAX_PLATFORMS", None)

N_CORES = 8
_PROGRAM_CACHE = {}
_last_results = None

# fraction of PSUM->SBUF copies issued on ScalarE (rest on VectorE)
_COPY_ACT_NUM = 1
_COPY_ACT_DEN = 1


def _install_ntff_shim():
    """Register the NTFF profile hook if the image's antenv lacks
    axon_hooks (needed only when tracing; harmless otherwise)."""
    import sys
    import types

    if "antenv.axon_hooks" in sys.modules:
        return
    try:
        import antenv
        from trn_agent_boot.trn_boot import _ntff_profile_via_ctypes

        mod = types.ModuleType("antenv.axon_hooks")
        mod._hook = None

        def set_axon_ntff_profile_hook(h):
            mod._hook = h

        def get_axon_ntff_profile_hook():
            return mod._hook

        mod.set_axon_ntff_profile_hook = set_axon_ntff_profile_hook
        mod.get_axon_ntff_profile_hook = get_axon_ntff_profile_hook
        sys.modules["antenv.axon_hooks"] = mod
        antenv.axon_hooks = mod
        hook = _ntff_profile_via_ctypes("/opt/axon/libaxon_pjrt.so")
        set_axon_ntff_profile_hook(hook)
    except Exception:
        pass


def _build_program(rl, S_c, C):
    import concourse.bacc as bacc
    import concourse.bass as bass
    import concourse.tile as tile
    from concourse import mybir

    G = len(rl)
    f32 = mybir.dt.float32
    bf16 = mybir.dt.bfloat16
    P = 128
    CH = C // P  # channel halves (2 for C=256)
    SUPER = 4  # j-tiles loaded per DMA (bigger transfers, fewer descriptors)
    n_full = S_c // P
    rem = S_c - n_full * P
    units = []
    j0 = 0
    while j0 < n_full:
        njj = min(SUPER, n_full - j0)
        units.append((j0 * P, njj, P))
        j0 += njj
    # remainder positions: pack several groups onto the partition dim so the
    # leftover 32-position tiles don't pay full-width instruction costs
    rem_units = []
    if rem:
        slots = P // rem
        for k in range((G + slots - 1) // slots):
            gs = list(range(k * slots, min((k + 1) * slots, G)))
            rem_units.append((k, gs))

    nc = bacc.Bacc(
        "TRN2", target_bir_lowering=False, debug=False, num_devices=N_CORES
    )
    xg_handles = [
        nc.dram_tensor(f"x{g}", [S_c, rl[g], C], f32, kind="ExternalInput")
        for g in range(G)
    ]
    ident_h = nc.dram_tensor("ident", [P, P], f32, kind="ExternalInput")
    xrem_handles = [
        nc.dram_tensor(
            f"xrem{k}",
            [len(gs) * rem, max(rl[g] for g in gs), C],
            f32,
            kind="ExternalInput",
        )
        for k, gs in rem_units
    ]
    out_h = nc.dram_tensor("out", [G, C, S_c], f32, kind="ExternalOutput")
    inv_sqrt = 1.0 / math.sqrt(C)
    Exp = mybir.ActivationFunctionType.Exp
    Square = mybir.ActivationFunctionType.Square
    Copy = mybir.ActivationFunctionType.Copy
    MULT = mybir.AluOpType.mult

    copy_counter = [0]

    def psum_copy(dst, src):
        # split PSUM->SBUF copies between ScalarE and VectorE
        c = copy_counter[0]
        copy_counter[0] += 1
        if c % _COPY_ACT_DEN < _COPY_ACT_NUM:
            nc.scalar.activation(dst, src, Copy)
        else:
            nc.vector.tensor_copy(dst, src)

    with tile.TileContext(nc) as tc:
        with (
            tc.tile_pool(name="xt", bufs=5) as xpool,
            tc.tile_pool(name="ones", bufs=1) as opool,
            tc.tile_pool(name="acc", bufs=4) as apool,
            tc.tile_pool(name="scr", bufs=4) as spool,
            tc.tile_pool(name="diag", bufs=6) as dpool,
            tc.tile_pool(name="small", bufs=12) as mpool,
            tc.tile_pool(name="psum", bufs=8, space="PSUM") as ppool,
        ):
            ident = opool.tile([P, P], f32)
            nc.sync.dma_start(out=ident[:], in_=ident_h.ap()[:])
            for k, gs in rem_units:
                nmax = max(rl[g] for g in gs)
                parts = len(gs) * rem
                X = xpool.tile([P, 1, nmax, C], f32, tag="xt")
                nc.sync.dma_start(out=X[:parts, 0], in_=xrem_handles[k].ap()[:])
                sc = mpool.tile([P, nmax], f32, tag="sc")
                scr = spool.tile([P, C], f32, tag="scr")
                nc.scalar.activation(
                    scr[:parts],
                    X[:parts, 0, 0, :],
                    Square,
                    scale=0.25,
                    accum_out=sc[:parts, 0:1],
                )
                scr2 = spool.tile([P, C], f32, tag="scr2")
                for m in range(1, nmax):
                    nc.vector.scalar_tensor_tensor(
                        out=scr2[:parts],
                        in0=X[:parts, 0, m, :],
                        scalar=inv_sqrt,
                        in1=X[:parts, 0, 0, :],
                        op0=MULT,
                        op1=MULT,
                        accum_out=sc[:parts, m : m + 1],
                    )
                u = mpool.tile([P, nmax], f32, tag="u")
                nc.scalar.activation(u[:parts], sc[:parts], Exp)
                # zero the zero-padded members (their exp(0)=1 would
                # pollute the softmax denominator)
                for si, g in enumerate(gs):
                    if rl[g] < nmax:
                        nc.vector.memset(
                            u[si * rem : (si + 1) * rem, rl[g] : nmax], 0.0
                        )
                z = mpool.tile([P, 1], f32, tag="z")
                nc.vector.reduce_sum(
                    out=z[:parts], in_=u[:parts], axis=mybir.AxisListType.X
                )
                rz = mpool.tile([P, 1], f32, tag="rz")
                nc.vector.reciprocal(rz[:parts], z[:parts])
                w = mpool.tile([P, nmax], f32, tag="w")
                nc.vector.tensor_scalar_mul(w[:parts], u[:parts], rz[:parts, 0:1])
                D = dpool.tile([P, nmax, P], f32, tag="D")
                ib = ident[:parts]
                i_rep = bass.AP(
                    tensor=ib.tensor,
                    offset=ib.offset,
                    ap=[ib.ap[0], [0, nmax], ib.ap[1]],
                )
                wb = w[:parts]
                w_rep = bass.AP(
                    tensor=wb.tensor,
                    offset=wb.offset,
                    ap=[wb.ap[0], wb.ap[1], [0, P]],
                )
                nc.vector.tensor_mul(D[:parts], i_rep, w_rep)
                ps = ppool.tile([P, CH, P], f32, tag="ps")
                acc = apool.tile([P, CH, 1, P], f32, tag="acc")
                for h in range(CH):
                    for m in range(nmax):
                        nc.tensor.matmul(
                            ps[:, h, :parts],
                            X[:parts, 0, m, h * P : (h + 1) * P],
                            D[:parts, m, :parts],
                            start=(m == 0),
                            stop=(m == nmax - 1),
                        )
                    psum_copy(acc[:, h, 0, :parts], ps[:, h, :parts])
                    for si, g in enumerate(gs):
                        dst = out_h.ap()[
                            g, h * P : (h + 1) * P, n_full * P : n_full * P + rem
                        ]
                        nc.gpsimd.dma_start(
                            out=dst, in_=acc[:, h, 0, si * rem : (si + 1) * rem]
                        )
            for g in range(G):
                n = rl[g]
                xg = xg_handles[g].ap()
                for p0, njj, parts in units:
                    X = xpool.tile([P, njj, n, C], f32, tag="xt")
                    src = xg[p0 : p0 + njj * parts].rearrange(
                        "(jj p) m c -> p jj m c", p=parts
                    )
                    nc.sync.dma_start(out=X[:parts], in_=src)
                    acc = apool.tile([P, CH, njj, P], f32, tag="acc")
                    for jj in range(njj):
                        sc = mpool.tile([P, n], f32, tag="sc")
                        scr = spool.tile([P, C], f32, tag="scr")
                        # ego score ||e||^2/sqrt(C) on ScalarE:
                        # Square(x/4) summed over c == sum(x^2)/16
                        nc.scalar.activation(
                            scr[:parts],
                            X[:parts, jj, 0, :],
                            Square,
                            scale=0.25,
                            accum_out=sc[:parts, 0:1],
                        )
                        scr2 = spool.tile([P, C], f32, tag="scr2")
                        for m in range(1, n):
                            # out = (x_m * 1/sqrt(C)) * ego; accum = per-
                            # partition sum -> the score, in one DVE pass
                            nc.vector.scalar_tensor_tensor(
                                out=scr2[:parts],
                                in0=X[:parts, jj, m, :],
                                scalar=inv_sqrt,
                                in1=X[:parts, jj, 0, :],
                                op0=MULT,
                                op1=MULT,
                                accum_out=sc[:parts, m : m + 1],
                            )
                        u = mpool.tile([P, n], f32, tag="u")
                        z = mpool.tile([P, 1], f32, tag="z")
                        nc.scalar.activation(
                            u[:parts], sc[:parts], Exp, accum_out=z[:parts]
                        )
                        rz = mpool.tile([P, 1], f32, tag="rz")
                        nc.vector.reciprocal(rz[:parts], z[:parts])
                        w = mpool.tile([P, n], f32, tag="w")
                        nc.vector.tensor_scalar_mul(
                            w[:parts], u[:parts], rz[:parts, 0:1]
                        )
                        # All n diagonals in one broadcast-AP tensor_tensor:
                        # D[p, m, j] = I[p, j] * w[p, m]
                        D = dpool.tile([P, n, P], f32, tag="D")
                        ib = ident[:parts]
                        i_rep = bass.AP(
                            tensor=ib.tensor,
                            offset=ib.offset,
                            ap=[ib.ap[0], [0, n], ib.ap[1]],
                        )
                        wb = w[:parts]
                        w_rep = bass.AP(
                            tensor=wb.tensor,
                            offset=wb.offset,
                            ap=[wb.ap[0], wb.ap[1], [0, P]],
                        )
                        nc.vector.tensor_mul(D[:parts], i_rep, w_rep)
                        ps = ppool.tile([P, CH, P], f32, tag="ps")
                        for h in range(CH):
                            for m in range(n):
                                # ctx^T accumulation on the TensorEngine:
                                # out[c, s] += X[s, c] * w_m[s]
                                nc.tensor.matmul(
                                    ps[:, h, :parts],
                                    X[:parts, jj, m, h * P : (h + 1) * P],
                                    D[:parts, m, :parts],
                                    start=(m == 0),
                                    stop=(m == n - 1),
                                )
                            psum_copy(acc[:, h, jj, :parts], ps[:, h, :parts])
                    for h in range(CH):
                        dst = out_h.ap()[
                            g, h * P : (h + 1) * P, p0 : p0 + njj * parts
                        ].rearrange("c (jj s) -> c jj s", s=parts)
                        # SWDGE: keeps the congested Sync sequencer free for
                        # input loads; the GpSimd engine is otherwise idle
                        nc.gpsimd.dma_start(out=dst, in_=acc[:, h, :, :parts])
    nc.compile()
    return nc


def kernel(x, record_len, fusion_method=None, **_ignored):
    global _last_results
    x = np.asarray(x, dtype=np.float32)
    rl = tuple(int(v) for v in np.asarray(record_len).reshape(-1))
    N, C, W, H = x.shape
    S = W * H
    assert S % N_CORES == 0, f"S={S} not divisible by {N_CORES}"
    S_c = S // N_CORES
    offs = np.concatenate([[0], np.cumsum(rl)]).astype(int)
    assert offs[-1] == N, f"record_len sums to {offs[-1]}, x has {N} rows"
    G = len(rl)

    if os.environ.get("BASS_TRACE"):
        _install_ntff_shim()

    key = (rl, S_c, C)
    nc = _PROGRAM_CACHE.get(key)
    if nc is None:
        nc = _build_program(rl, S_c, C)
        _PROGRAM_CACHE[key] = nc

    xs = x.reshape(N, C, S)
    ident = np.eye(128, dtype=np.float32)
    P = 128
    n_full = S_c // P
    rem = S_c - n_full * P
    rem_units = []
    if rem:
        slots = P // rem
        for k in range((G + slots - 1) // slots):
            rem_units.append((k, list(range(k * slots, min((k + 1) * slots, G)))))
    in_maps = []
    for i in range(N_CORES):
        sl = xs[:, :, i * S_c : (i + 1) * S_c]
        m = {
            f"x{g}": np.ascontiguousarray(
                sl[offs[g] : offs[g + 1]].transpose(2, 0, 1)
            )
            for g in range(G)
        }
        m["ident"] = ident
        for k, gs in rem_units:
            nmax = max(rl[g] for g in gs)
            pk = np.zeros((len(gs) * rem, nmax, C), dtype=np.float32)
            for si, g in enumerate(gs):
                blk = sl[offs[g] : offs[g + 1], :, n_full * P : n_full * P + rem]
                pk[si * rem : (si + 1) * rem, : rl[g]] = blk.transpose(2, 0, 1)
            m[f"xrem{k}"] = pk
        in_maps.append(m)

    from concourse.bass_utils import run_bass_kernel_spmd

    res = run_bass_kernel_spmd(nc, in_maps, list(range(N_CORES)))
    _last_results = res

    out = np.empty((G, C, S), dtype=np.float32)
    for i in range(N_CORES):
        # per-core result is already [G, C, S_c]
        out[:, :, i * S_c : (i + 1) * S_c] = np.asarray(res.results[i]["out"])
    return out.reshape(G, C, W, H)


# revision 16
# speedup vs baseline: 1.0163x; 1.0163x over previous
"""AttFusion (attentive CAV fusion) Trainium2 kernel.

Reference semantics per ragged group of n rows (n = record_len[g]):
  tokens q[s, m, :] = x[row m, :, s]  (s over W*H spatial positions)
  score[s, m] = <q[s, 0, :], q[s, m, :]> / sqrt(C)
  w = softmax over m; out[g, :, s] = sum_m w[s, m] * q[s, m, :]

Strategy: data-parallel over spatial positions across 8 NeuronCores
(every core processes all groups over S/8 positions — SPMD-uniform and
load-balanced). Host side transposes each group's rows to
position-major [S_c, n, C] so the device kernel sees SBUF tiles with
partition = position, free = (member, channel). Per 128-position tile:
  - ego score ||e||^2/sqrt(C): ScalarE Square(scale=1/4) with accum_out
  - member scores: one fused DVE scalar_tensor_tensor per member
    (product + free-dim reduce + 1/sqrt(C) scale in one instruction)
  - softmax: ScalarE Exp with accum_out, DVE reciprocal; no
    max-subtraction (scores bounded ~ sqrt(C)+few sigma, far from
    fp32 overflow); weights w = u/Z normalized up front (bf16-safe:
    the dominant ego weight rounds to exactly 1.0)
  - the weighted sum rides the otherwise-idle TensorEngine *flipped*:
    lhsT = X c-half (fp32 stationary), rhs = diag(w_m) (bf16 moving,
    single-pass N=128 matmuls) accumulated over members in PSUM.
    All n diagonals are built in ONE broadcast-AP tensor_tensor
    (identity-replicated x weight-replicated). PSUM holds ctx in
    [c, s] layout = the output layout, so the host-side gather needs
    no transpose. PSUM->SBUF copies are split ScalarE/VectorE.
"""

import math
import os

import numpy as np

_jp = os.environ.get("JAX_PLATFORMS")
if _jp is not None and "axon" not in _jp:
    # The device run goes through the axon PJRT backend; a cpu-only pin
    # (used for reference-side jax) would hide the NeuronCores.
    os.environ.pop("JAX_PLATFORMS", None)

N_CORES = 8
_PROGRAM_CACHE = {}
_last_results = None

# fraction of PSUM->SBUF copies issued on ScalarE (rest on VectorE)
_COPY_ACT_NUM = 4
_COPY_ACT_DEN = 5


def _install_ntff_shim():
    """Register the NTFF profile hook if the image's antenv lacks
    axon_hooks (needed only when tracing; harmless otherwise)."""
    import sys
    import types

    if "antenv.axon_hooks" in sys.modules:
        return
    try:
        import antenv
        from trn_agent_boot.trn_boot import _ntff_profile_via_ctypes

        mod = types.ModuleType("antenv.axon_hooks")
        mod._hook = None

        def set_axon_ntff_profile_hook(h):
            mod._hook = h

        def get_axon_ntff_profile_hook():
            return mod._hook

        mod.set_axon_ntff_profile_hook = set_axon_ntff_profile_hook
        mod.get_axon_ntff_profile_hook = get_axon_ntff_profile_hook
        sys.modules["antenv.axon_hooks"] = mod
        antenv.axon_hooks = mod
        hook = _ntff_profile_via_ctypes("/opt/axon/libaxon_pjrt.so")
        set_axon_ntff_profile_hook(hook)
    except Exception:
        pass


def _build_program(rl, S_c, C):
    import concourse.bacc as bacc
    import concourse.bass as bass
    import concourse.tile as tile
    from concourse import mybir

    G = len(rl)
    f32 = mybir.dt.float32
    bf16 = mybir.dt.bfloat16
    P = 128
    CH = C // P  # channel halves (2 for C=256)
    SUPER = 4  # j-tiles loaded per DMA (bigger transfers, fewer descriptors)
    n_full = S_c // P
    rem = S_c - n_full * P
    units = []
    j0 = 0
    while j0 < n_full:
        njj = min(SUPER, n_full - j0)
        units.append((j0 * P, njj, P))
        j0 += njj
    # remainder positions: pack several groups onto the partition dim so the
    # leftover 32-position tiles don't pay full-width instruction costs
    rem_units = []
    if rem:
        slots = P // rem
        for k in range((G + slots - 1) // slots):
            gs = list(range(k * slots, min((k + 1) * slots, G)))
            rem_units.append((k, gs))

    nc = bacc.Bacc(
        "TRN2", target_bir_lowering=False, debug=False, num_devices=N_CORES
    )
    xg_handles = [
        nc.dram_tensor(f"x{g}", [S_c, rl[g], C], f32, kind="ExternalInput")
        for g in range(G)
    ]
    ident_h = nc.dram_tensor("ident", [P, P], f32, kind="ExternalInput")
    xrem_handles = [
        nc.dram_tensor(
            f"xrem{k}",
            [len(gs) * rem, max(rl[g] for g in gs), C],
            f32,
            kind="ExternalInput",
        )
        for k, gs in rem_units
    ]
    out_h = nc.dram_tensor("out", [G, C, S_c], f32, kind="ExternalOutput")
    inv_sqrt = 1.0 / math.sqrt(C)
    Exp = mybir.ActivationFunctionType.Exp
    Square = mybir.ActivationFunctionType.Square
    Copy = mybir.ActivationFunctionType.Copy
    MULT = mybir.AluOpType.mult

    copy_counter = [0]

    def psum_copy(dst, src):
        # split PSUM->SBUF copies between ScalarE and VectorE
        c = copy_counter[0]
        copy_counter[0] += 1
        if c % _COPY_ACT_DEN < _COPY_ACT_NUM:
            nc.scalar.activation(dst, src, Copy)
        else:
            nc.vector.tensor_copy(dst, src)

    with tile.TileContext(nc) as tc:
        with (
            tc.tile_pool(name="xt", bufs=5) as xpool,
            tc.tile_pool(name="ones", bufs=1) as opool,
            tc.tile_pool(name="acc", bufs=4) as apool,
            tc.tile_pool(name="scr", bufs=4) as spool,
            tc.tile_pool(name="diag", bufs=6) as dpool,
            tc.tile_pool(name="small", bufs=12) as mpool,
            tc.tile_pool(name="psum", bufs=8, space="PSUM") as ppool,
        ):
            ident = opool.tile([P, P], f32)
            nc.sync.dma_start(out=ident[:], in_=ident_h.ap()[:])
            for k, gs in rem_units:
                nmax = max(rl[g] for g in gs)
                parts = len(gs) * rem
                X = xpool.tile([P, 1, nmax, C], f32, tag="xt")
                nc.sync.dma_start(out=X[:parts, 0], in_=xrem_handles[k].ap()[:])
                sc = mpool.tile([P, nmax], f32, tag="sc")
                scr = spool.tile([P, C], f32, tag="scr")
                nc.scalar.activation(
                    scr[:parts],
                    X[:parts, 0, 0, :],
                    Square,
                    scale=0.25,
                    accum_out=sc[:parts, 0:1],
                )
                scr2 = spool.tile([P, C], f32, tag="scr2")
                for m in range(1, nmax):
                    nc.vector.scalar_tensor_tensor(
                        out=scr2[:parts],
                        in0=X[:parts, 0, m, :],
                        scalar=inv_sqrt,
                        in1=X[:parts, 0, 0, :],
                        op0=MULT,
                        op1=MULT,
                        accum_out=sc[:parts, m : m + 1],
                    )
                u = mpool.tile([P, nmax], f32, tag="u")
                nc.scalar.activation(u[:parts], sc[:parts], Exp)
                # zero the zero-padded members (their exp(0)=1 would
                # pollute the softmax denominator)
                for si, g in enumerate(gs):
                    if rl[g] < nmax:
                        nc.vector.memset(
                            u[si * rem : (si + 1) * rem, rl[g] : nmax], 0.0
                        )
                z = mpool.tile([P, 1], f32, tag="z")
                nc.vector.reduce_sum(
                    out=z[:parts], in_=u[:parts], axis=mybir.AxisListType.X
                )
                rz = mpool.tile([P, 1], f32, tag="rz")
                nc.vector.reciprocal(rz[:parts], z[:parts])
                w = mpool.tile([P, nmax], f32, tag="w")
                nc.vector.tensor_scalar_mul(w[:parts], u[:parts], rz[:parts, 0:1])
                D = dpool.tile([P, nmax, P], f32, tag="D")
                ib = ident[:parts]
                i_rep = bass.AP(
                    tensor=ib.tensor,
                    offset=ib.offset,
                    ap=[ib.ap[0], [0, nmax], ib.ap[1]],
                )
                wb = w[:parts]
                w_rep = bass.AP(
                    tensor=wb.tensor,
                    offset=wb.offset,
                    ap=[wb.ap[0], wb.ap[1], [0, P]],
                )
                nc.vector.tensor_mul(D[:parts], i_rep, w_rep)
                ps = ppool.tile([P, CH, P], f32, tag="ps")
                acc = apool.tile([P, CH, 1, P], f32, tag="acc")
                for h in range(CH):
                    for m in range(nmax):
                        nc.tensor.matmul(
                            ps[:, h, :parts],
                            X[:parts, 0, m, h * P : (h + 1) * P],
                            D[:parts, m, :parts],
                            start=(m == 0),
                            stop=(m == nmax - 1),
                        )
                    psum_copy(acc[:, h, 0, :parts], ps[:, h, :parts])
                    for si, g in enumerate(gs):
                        dst = out_h.ap()[
                            g, h * P : (h + 1) * P, n_full * P : n_full * P + rem
                        ]
                        nc.gpsimd.dma_start(
                            out=dst, in_=acc[:, h, 0, si * rem : (si + 1) * rem]
                        )
            taper = [4, 2, 1, 1]
            for g in range(G):
                n = rl[g]
                xg = xg_handles[g].ap()
                g_units = units
                if g == G - 1 and rem == 0 or g == G - 1:
                    g_units = []
                    j0t = 0
                    for njj_t in taper:
                        if j0t >= n_full:
                            break
                        njj_t = min(njj_t, n_full - j0t)
                        g_units.append((j0t * P, njj_t, P))
                        j0t += njj_t
                    while j0t < n_full:
                        njj_t = min(1, n_full - j0t)
                        g_units.append((j0t * P, njj_t, P))
                        j0t += njj_t
                for p0, njj, parts in g_units:
                    X = xpool.tile([P, njj, n, C], f32, tag="xt")
                    src = xg[p0 : p0 + njj * parts].rearrange(
                        "(jj p) m c -> p jj m c", p=parts
                    )
                    nc.sync.dma_start(out=X[:parts], in_=src)
                    acc = apool.tile([P, CH, njj, P], f32, tag="acc")
                    for jj in range(njj):
                        sc = mpool.tile([P, n], f32, tag="sc")
                        scr = spool.tile([P, C], f32, tag="scr")
                        # ego score ||e||^2/sqrt(C) on ScalarE:
                        # Square(x/4) summed over c == sum(x^2)/16
                        nc.scalar.activation(
                            scr[:parts],
                            X[:parts, jj, 0, :],
                            Square,
                            scale=0.25,
                            accum_out=sc[:parts, 0:1],
                        )
                        scr2 = spool.tile([P, C], f32, tag="scr2")
                        for m in range(1, n):
                            # out = (x_m * 1/sqrt(C)) * ego; accum = per-
                            # partition sum -> the score, in one DVE pass
                            nc.vector.scalar_tensor_tensor(
                                out=scr2[:parts],
                                in0=X[:parts, jj, m, :],
                                scalar=inv_sqrt,
                                in1=X[:parts, jj, 0, :],
                                op0=MULT,
                                op1=MULT,
                                accum_out=sc[:parts, m : m + 1],
                            )
                        u = mpool.tile([P, n], f32, tag="u")
                        z = mpool.tile([P, 1], f32, tag="z")
                        nc.scalar.activation(
                            u[:parts], sc[:parts], Exp, accum_out=z[:parts]
                        )
                        rz = mpool.tile([P, 1], f32, tag="rz")
                        nc.vector.reciprocal(rz[:parts], z[:parts])
                        w = mpool.tile([P, n], f32, tag="w")
                        nc.vector.tensor_scalar_mul(
                            w[:parts], u[:parts], rz[:parts, 0:1]
                        )
                        # All n diagonals in one broadcast-AP tensor_tensor:
                        # D[p, m, j] = I[p, j] * w[p, m]
                        D = dpool.tile([P, n, P], f32, tag="D")
                        ib = ident[:parts]
                        i_rep = bass.AP(
                            tensor=ib.tensor,
                            offset=ib.offset,
                            ap=[ib.ap[0], [0, n], ib.ap[1]],
                        )
                        wb = w[:parts]
                        w_rep = bass.AP(
                            tensor=wb.tensor,
                            offset=wb.offset,
                            ap=[wb.ap[0], wb.ap[1], [0, P]],
                        )
                        nc.vector.tensor_mul(D[:parts], i_rep, w_rep)
                        ps = ppool.tile([P, CH, P], f32, tag="ps")
                        for h in range(CH):
                            for m in range(n):
                                # ctx^T accumulation on the TensorEngine:
                                # out[c, s] += X[s, c] * w_m[s]
                                nc.tensor.matmul(
                                    ps[:, h, :parts],
                                    X[:parts, jj, m, h * P : (h + 1) * P],
                                    D[:parts, m, :parts],
                                    start=(m == 0),
                                    stop=(m == n - 1),
                                )
                            psum_copy(acc[:, h, jj, :parts], ps[:, h, :parts])
                    for h in range(CH):
                        dst = out_h.ap()[
                            g, h * P : (h + 1) * P, p0 : p0 + njj * parts
                        ].rearrange("c (jj s) -> c jj s", s=parts)
                        # SWDGE: keeps the congested Sync sequencer free for
                        # input loads; the GpSimd engine is otherwise idle
                        nc.gpsimd.dma_start(out=dst, in_=acc[:, h, :, :parts])
    nc.compile()
    return nc


def kernel(x, record_len, fusion_method=None, **_ignored):
    global _last_results
    x = np.asarray(x, dtype=np.float32)
    rl = tuple(int(v) for v in np.asarray(record_len).reshape(-1))
    N, C, W, H = x.shape
    S = W * H
    assert S % N_CORES == 0, f"S={S} not divisible by {N_CORES}"
    S_c = S // N_CORES
    offs = np.concatenate([[0], np.cumsum(rl)]).astype(int)
    assert offs[-1] == N, f"record_len sums to {offs[-1]}, x has {N} rows"
    G = len(rl)

    if os.environ.get("BASS_TRACE"):
        _install_ntff_shim()

    key = (rl, S_c, C)
    nc = _PROGRAM_CACHE.get(key)
    if nc is None:
        nc = _build_program(rl, S_c, C)
        _PROGRAM_CACHE[key] = nc

    xs = x.reshape(N, C, S)
    ident = np.eye(128, dtype=np.float32)
    P = 128
    n_full = S_c // P
    rem = S_c - n_full * P
    rem_units = []
    if rem:
        slots = P // rem
        for k in range((G + slots - 1) // slots):
            rem_units.append((k, list(range(k * slots, min((k + 1) * slots, G)))))
    in_maps = []
    for i in range(N_CORES):
        sl = xs[:, :, i * S_c : (i + 1) * S_c]
        m = {
            f"x{g}": np.ascontiguousarray(
                sl[offs[g] : offs[g + 1]].transpose(2, 0, 1)
            )
            for g in range(G)
        }
        m["ident"] = ident
        for k, gs in rem_units:
            nmax = max(rl[g] for g in gs)
            pk = np.zeros((len(gs) * rem, nmax, C), dtype=np.float32)
            for si, g in enumerate(gs):
                blk = sl[offs[g] : offs[g + 1], :, n_full * P : n_full * P + rem]
                pk[si * rem : (si + 1) * rem, : rl[g]] = blk.transpose(2, 0, 1)
            m[f"xrem{k}"] = pk
        in_maps.append(m)

    from concourse.bass_utils import run_bass_kernel_spmd

    res = run_bass_kernel_spmd(nc, in_maps, list(range(N_CORES)))
    _last_results = res

    out = np.empty((G, C, S), dtype=np.float32)
    for i in range(N_CORES):
        # per-core result is already [G, C, S_c]
        out[:, :, i * S_c : (i + 1) * S_c] = np.asarray(res.results[i]["out"])
    return out.reshape(G, C, W, H)


# revision 17
# speedup vs baseline: 1.2152x; 1.1957x over previous
"""AttFusion (attentive CAV fusion) Trainium2 kernel.

Reference semantics per ragged group of n rows (n = record_len[g]):
  tokens q[s, m, :] = x[row m, :, s]  (s over W*H spatial positions)
  score[s, m] = <q[s, 0, :], q[s, m, :]> / sqrt(C)
  w = softmax over m; out[g, :, s] = sum_m w[s, m] * q[s, m, :]

Strategy: data-parallel over spatial positions across 8 NeuronCores
(every core processes all groups over S/8 positions — SPMD-uniform and
load-balanced). Host side transposes each group's rows to
position-major [S_c, n, C] so the device kernel sees SBUF tiles with
partition = position, free = (member, channel). Per 128-position tile:
  - ego score ||e||^2/sqrt(C): ScalarE Square(scale=1/4) with accum_out
  - member scores: one fused DVE scalar_tensor_tensor per member
    (product + free-dim reduce + 1/sqrt(C) scale in one instruction)
  - softmax: ScalarE Exp with accum_out, DVE reciprocal; no
    max-subtraction (scores bounded ~ sqrt(C)+few sigma, far from
    fp32 overflow); weights w = u/Z normalized up front (bf16-safe:
    the dominant ego weight rounds to exactly 1.0)
  - the weighted sum rides the otherwise-idle TensorEngine *flipped*:
    lhsT = X c-half (fp32 stationary), rhs = diag(w_m) (bf16 moving,
    single-pass N=128 matmuls) accumulated over members in PSUM.
    All n diagonals are built in ONE broadcast-AP tensor_tensor
    (identity-replicated x weight-replicated). PSUM holds ctx in
    [c, s] layout = the output layout, so the host-side gather needs
    no transpose. PSUM->SBUF copies are split ScalarE/VectorE.
"""

import math
import os

import numpy as np

_jp = os.environ.get("JAX_PLATFORMS")
if _jp is not None and "axon" not in _jp:
    # The device run goes through the axon PJRT backend; a cpu-only pin
    # (used for reference-side jax) would hide the NeuronCores.
    os.environ.pop("JAX_PLATFORMS", None)

N_CORES = 8
_PROGRAM_CACHE = {}
_last_results = None

# fraction of PSUM->SBUF copies issued on ScalarE (rest on VectorE)
_COPY_ACT_NUM = 4
_COPY_ACT_DEN = 5


def _install_ntff_shim():
    """Register the NTFF profile hook if the image's antenv lacks
    axon_hooks (needed only when tracing; harmless otherwise)."""
    import sys
    import types

    if "antenv.axon_hooks" in sys.modules:
        return
    try:
        import antenv
        from trn_agent_boot.trn_boot import _ntff_profile_via_ctypes

        mod = types.ModuleType("antenv.axon_hooks")
        mod._hook = None

        def set_axon_ntff_profile_hook(h):
            mod._hook = h

        def get_axon_ntff_profile_hook():
            return mod._hook

        mod.set_axon_ntff_profile_hook = set_axon_ntff_profile_hook
        mod.get_axon_ntff_profile_hook = get_axon_ntff_profile_hook
        sys.modules["antenv.axon_hooks"] = mod
        antenv.axon_hooks = mod
        hook = _ntff_profile_via_ctypes("/opt/axon/libaxon_pjrt.so")
        set_axon_ntff_profile_hook(hook)
    except Exception:
        pass


def _build_program(rl, S_c, C):
    import concourse.bacc as bacc
    import concourse.bass as bass
    import concourse.tile as tile
    from concourse import mybir

    G = len(rl)
    f32 = mybir.dt.float32
    bf16 = mybir.dt.bfloat16
    P = 128
    CH = C // P  # channel halves (2 for C=256)
    SUPER = 4  # j-tiles loaded per DMA (bigger transfers, fewer descriptors)
    n_full = S_c // P
    rem = S_c - n_full * P
    units = []
    j0 = 0
    while j0 < n_full:
        njj = min(SUPER, n_full - j0)
        units.append((j0 * P, njj, P))
        j0 += njj
    # remainder positions: pack several groups onto the partition dim so the
    # leftover 32-position tiles don't pay full-width instruction costs
    rem_units = []
    if rem:
        slots = P // rem
        for k in range((G + slots - 1) // slots):
            gs = list(range(k * slots, min((k + 1) * slots, G)))
            rem_units.append((k, gs))

    nc = bacc.Bacc(
        "TRN2", target_bir_lowering=False, debug=False, num_devices=N_CORES
    )
    xg_handles = [
        nc.dram_tensor(f"x{g}", [S_c, rl[g], C], f32, kind="ExternalInput")
        for g in range(G)
    ]
    ident_h = nc.dram_tensor("ident", [P, P], f32, kind="ExternalInput")
    xrem_handles = [
        nc.dram_tensor(
            f"xrem{k}",
            [len(gs) * rem, max(rl[g] for g in gs), C],
            f32,
            kind="ExternalInput",
        )
        for k, gs in rem_units
    ]
    out_h = nc.dram_tensor("out", [G, C, S_c], f32, kind="ExternalOutput")
    inv_sqrt = 1.0 / math.sqrt(C)
    Exp = mybir.ActivationFunctionType.Exp
    Square = mybir.ActivationFunctionType.Square
    Copy = mybir.ActivationFunctionType.Copy
    MULT = mybir.AluOpType.mult

    copy_counter = [0]

    def psum_copy(dst, src):
        # split PSUM->SBUF copies between ScalarE and VectorE
        c = copy_counter[0]
        copy_counter[0] += 1
        if c % _COPY_ACT_DEN < _COPY_ACT_NUM:
            nc.scalar.activation(dst, src, Copy)
        else:
            nc.vector.tensor_copy(dst, src)

    with tile.TileContext(nc) as tc:
        with (
            tc.tile_pool(name="xt", bufs=6) as xpool,
            tc.tile_pool(name="ones", bufs=1) as opool,
            tc.tile_pool(name="acc", bufs=4) as apool,
            tc.tile_pool(name="scr", bufs=4) as spool,
            tc.tile_pool(name="diag", bufs=6) as dpool,
            tc.tile_pool(name="small", bufs=12) as mpool,
            tc.tile_pool(name="psum", bufs=8, space="PSUM") as ppool,
        ):
            ident = opool.tile([P, P], f32)
            nc.sync.dma_start(out=ident[:], in_=ident_h.ap()[:])
            for k, gs in rem_units:
                nmax = max(rl[g] for g in gs)
                parts = len(gs) * rem
                X = xpool.tile([P, 1, nmax, C], f32, tag="xt")
                nc.sync.dma_start(out=X[:parts, 0], in_=xrem_handles[k].ap()[:])
                sc = mpool.tile([P, nmax], f32, tag="sc")
                scr = spool.tile([P, C], f32, tag="scr")
                nc.scalar.activation(
                    scr[:parts],
                    X[:parts, 0, 0, :],
                    Square,
                    scale=0.25,
                    accum_out=sc[:parts, 0:1],
                )
                scr2 = spool.tile([P, C], f32, tag="scr2")
                for m in range(1, nmax):
                    nc.vector.scalar_tensor_tensor(
                        out=scr2[:parts],
                        in0=X[:parts, 0, m, :],
                        scalar=inv_sqrt,
                        in1=X[:parts, 0, 0, :],
                        op0=MULT,
                        op1=MULT,
                        accum_out=sc[:parts, m : m + 1],
                    )
                u = mpool.tile([P, nmax], f32, tag="u")
                nc.scalar.activation(u[:parts], sc[:parts], Exp)
                # zero the zero-padded members (their exp(0)=1 would
                # pollute the softmax denominator)
                for si, g in enumerate(gs):
                    if rl[g] < nmax:
                        nc.vector.memset(
                            u[si * rem : (si + 1) * rem, rl[g] : nmax], 0.0
                        )
                z = mpool.tile([P, 1], f32, tag="z")
                nc.vector.reduce_sum(
                    out=z[:parts], in_=u[:parts], axis=mybir.AxisListType.X
                )
                rz = mpool.tile([P, 1], f32, tag="rz")
                nc.vector.reciprocal(rz[:parts], z[:parts])
                w = mpool.tile([P, nmax], f32, tag="w")
                nc.vector.tensor_scalar_mul(w[:parts], u[:parts], rz[:parts, 0:1])
                D = dpool.tile([P, nmax, P], f32, tag="D")
                ib = ident[:parts]
                i_rep = bass.AP(
                    tensor=ib.tensor,
                    offset=ib.offset,
                    ap=[ib.ap[0], [0, nmax], ib.ap[1]],
                )
                wb = w[:parts]
                w_rep = bass.AP(
                    tensor=wb.tensor,
                    offset=wb.offset,
                    ap=[wb.ap[0], wb.ap[1], [0, P]],
                )
                nc.vector.tensor_mul(D[:parts], i_rep, w_rep)
                ps = ppool.tile([P, CH, P], f32, tag="ps")
                acc = apool.tile([P, CH, 1, P], f32, tag="acc")
                for h in range(CH):
                    for m in range(nmax):
                        nc.tensor.matmul(
                            ps[:, h, :parts],
                            X[:parts, 0, m, h * P : (h + 1) * P],
                            D[:parts, m, :parts],
                            start=(m == 0),
                            stop=(m == nmax - 1),
                        )
                    psum_copy(acc[:, h, 0, :parts], ps[:, h, :parts])
                    for si, g in enumerate(gs):
                        dst = out_h.ap()[
                            g, h * P : (h + 1) * P, n_full * P : n_full * P + rem
                        ]
                        nc.gpsimd.dma_start(
                            out=dst, in_=acc[:, h, 0, si * rem : (si + 1) * rem]
                        )
            for g in range(G):
                n = rl[g]
                xg = xg_handles[g].ap()
                for p0, njj, parts in units:
                    X = xpool.tile([P, njj, n, C], f32, tag="xt")
                    src = xg[p0 : p0 + njj * parts].rearrange(
                        "(jj p) m c -> p jj m c", p=parts
                    )
                    nc.sync.dma_start(out=X[:parts], in_=src)
                    acc = apool.tile([P, CH, njj, P], f32, tag="acc")
                    for jj in range(njj):
                        sc = mpool.tile([P, n], f32, tag="sc")
                        scr = spool.tile([P, C], f32, tag="scr")
                        # ego score ||e||^2/sqrt(C) on ScalarE:
                        # Square(x/4) summed over c == sum(x^2)/16
                        nc.scalar.activation(
                            scr[:parts],
                            X[:parts, jj, 0, :],
                            Square,
                            scale=0.25,
                            accum_out=sc[:parts, 0:1],
                        )
                        scr2 = spool.tile([P, C], f32, tag="scr2")
                        for m in range(1, n):
                            # out = (x_m * 1/sqrt(C)) * ego; accum = per-
                            # partition sum -> the score, in one DVE pass
                            nc.vector.scalar_tensor_tensor(
                                out=scr2[:parts],
                                in0=X[:parts, jj, m, :],
                                scalar=inv_sqrt,
                                in1=X[:parts, jj, 0, :],
                                op0=MULT,
                                op1=MULT,
                                accum_out=sc[:parts, m : m + 1],
                            )
                        u = mpool.tile([P, n], f32, tag="u")
                        z = mpool.tile([P, 1], f32, tag="z")
                        nc.scalar.activation(
                            u[:parts], sc[:parts], Exp, accum_out=z[:parts]
                        )
                        rz = mpool.tile([P, 1], f32, tag="rz")
                        nc.vector.reciprocal(rz[:parts], z[:parts])
                        w = mpool.tile([P, n], f32, tag="w")
                        nc.vector.tensor_scalar_mul(
                            w[:parts], u[:parts], rz[:parts, 0:1]
                        )
                        # All n diagonals in one broadcast-AP tensor_tensor:
                        # D[p, m, j] = I[p, j] * w[p, m]
                        D = dpool.tile([P, n, P], f32, tag="D")
                        ib = ident[:parts]
                        i_rep = bass.AP(
                            tensor=ib.tensor,
                            offset=ib.offset,
                            ap=[ib.ap[0], [0, n], ib.ap[1]],
                        )
                        wb = w[:parts]
                        w_rep = bass.AP(
                            tensor=wb.tensor,
                            offset=wb.offset,
                            ap=[wb.ap[0], wb.ap[1], [0, P]],
                        )
                        nc.vector.tensor_mul(D[:parts], i_rep, w_rep)
                        ps = ppool.tile([P, CH, P], f32, tag="ps")
                        for h in range(CH):
                            for m in range(n):
                                # ctx^T accumulation on the TensorEngine:
                                # out[c, s] += X[s, c] * w_m[s]
                                nc.tensor.matmul(
                                    ps[:, h, :parts],
                                    X[:parts, jj, m, h * P : (h + 1) * P],
                                    D[:parts, m, :parts],
                                    start=(m == 0),
                                    stop=(m == n - 1),
                                )
                            psum_copy(acc[:, h, jj, :parts], ps[:, h, :parts])
                    for h in range(CH):
                        dst = out_h.ap()[
                            g, h * P : (h + 1) * P, p0 : p0 + njj * parts
                        ].rearrange("c (jj s) -> c jj s", s=parts)
                        # SWDGE: keeps the congested Sync sequencer free for
                        # input loads; the GpSimd engine is otherwise idle
                        nc.gpsimd.dma_start(out=dst, in_=acc[:, h, :, :parts])
    nc.compile()
    return nc


def kernel(x, record_len, fusion_method=None, **_ignored):
    global _last_results
    x = np.asarray(x, dtype=np.float32)
    rl = tuple(int(v) for v in np.asarray(record_len).reshape(-1))
    N, C, W, H = x.shape
    S = W * H
    assert S % N_CORES == 0, f"S={S} not divisible by {N_CORES}"
    S_c = S // N_CORES
    offs = np.concatenate([[0], np.cumsum(rl)]).astype(int)
    assert offs[-1] == N, f"record_len sums to {offs[-1]}, x has {N} rows"
    G = len(rl)

    if os.environ.get("BASS_TRACE"):
        _install_ntff_shim()

    key = (rl, S_c, C)
    nc = _PROGRAM_CACHE.get(key)
    if nc is None:
        nc = _build_program(rl, S_c, C)
        _PROGRAM_CACHE[key] = nc

    xs = x.reshape(N, C, S)
    ident = np.eye(128, dtype=np.float32)
    P = 128
    n_full = S_c // P
    rem = S_c - n_full * P
    rem_units = []
    if rem:
        slots = P // rem
        for k in range((G + slots - 1) // slots):
            rem_units.append((k, list(range(k * slots, min((k + 1) * slots, G)))))
    in_maps = []
    for i in range(N_CORES):
        sl = xs[:, :, i * S_c : (i + 1) * S_c]
        m = {
            f"x{g}": np.ascontiguousarray(
                sl[offs[g] : offs[g + 1]].transpose(2, 0, 1)
            )
            for g in range(G)
        }
        m["ident"] = ident
        for k, gs in rem_units:
            nmax = max(rl[g] for g in gs)
            pk = np.zeros((len(gs) * rem, nmax, C), dtype=np.float32)
            for si, g in enumerate(gs):
                blk = sl[offs[g] : offs[g + 1], :, n_full * P : n_full * P + rem]
                pk[si * rem : (si + 1) * rem, : rl[g]] = blk.transpose(2, 0, 1)
            m[f"xrem{k}"] = pk
        in_maps.append(m)

    from concourse.bass_utils import run_bass_kernel_spmd

    res = run_bass_kernel_spmd(nc, in_maps, list(range(N_CORES)))
    _last_results = res

    out = np.empty((G, C, S), dtype=np.float32)
    for i in range(N_CORES):
        # per-core result is already [G, C, S_c]
        out[:, :, i * S_c : (i + 1) * S_c] = np.asarray(res.results[i]["out"])
    return out.reshape(G, C, W, H)
